# revision 20
# baseline (speedup 1.0000x reference)
"""CapsNet dynamic-routing layer on 8 Trainium2 NeuronCores.

Strategy (v2)
-------------
Shard the R=512 routes across 8 cores (64 each); W is read exactly once
machine-wide. Per core:

Phase A — u_hat production:
  u_hat[b,r,c,o] = sum_i W[r,c,o,i] x[b,r,i] via TensorE: stationary =
  x[r] [I=128, B=32] fp16 hi/lo, moving = W[r] [I, co-chunk 512] fp16
  hi/lo, 3 passes (hh+lh+hl) PSUM-accumulated -> ~fp32 precision.
  W is re-laid out host-side so each DMA is a large near-linear block
  (1 MB hi / 0.5 MB lo), issued on the two HWDGE queues (Sync + Act)
  with just-in-time double buffering -> big descriptors, ~HBM-rate DMA.
  Iter-0 s rides the evacuation path (DVE adds); its AllReduce is split
  in two route-halves so the first AR hides under phase A.

Phase B — routing iterations, chunked pipeline:
  a = <u, v>: DVE mult + reduce over o, in 4-capsule chunks.
  s = sum_r c*u: DVE mult (contiguous write) then route-fold on TensorE
  via replicated-eye delta matmuls PSUM-accumulated over the 16
  r-groups (no DVE reduce). Each c-octant's AllReduce (64 KB) launches
  as soon as its fold lands; squash + v-broadcast per octant gate the
  next iteration's a-chunks, so AR latency pipelines under compute.

Numerics: fp16 hi+lo splits carry ~22 mantissa bits; routing amplifies
u_hat error ~500x so everything in the routing loop stays fp32.
"""
import sys

sys.path.insert(0, "/opt/trn_rl_repo")

import numpy as np

import concourse.bass as bass
import concourse.tile as tile
from concourse import mybir
from concourse.bass_utils import run_bass_kernel_spmd

F16 = mybir.dt.float16
F32 = mybir.dt.float32

NCORES = 8
B, R, C, O, I = 32, 512, 32, 64, 128
CO = C * O                # 2048
RL = R // NCORES          # 64 routes per core
J = 4                     # col-strips (rj) in u layout
G = RL // J               # 16 r-groups in u layout
P2 = RL // 2              # 32 two-route W tiles
NQ = 4                    # co octants (AR chunks)
Q = CO // NQ              # 512
HQ = Q // 2               # half-octant (4 capsules)
EPS = 1e-8

_cache = {}


def _legalize_install(nc):
    """This walrus build accepts at most one sync wait per instruction and
    none on Matmult; hoist extras onto standalone EventSemaphore ops."""
    import json
    from concourse import mybir as _mb

    def legalize(raw: bytes) -> bytes:
        d = json.loads(raw)
        ctr = 0
        for f in d.get("functions", []):
            for blk in f.get("blocks", []):
                out = []
                for ins in blk.get("instructions", []):
                    si = ins.get("sync_info")
                    waits = (si or {}).get("on_wait") or []
                    keep = 0 if ins.get("opcode") in ("Matmult", "Ldweights") else 1
                    if len(waits) > keep:
                        nh = len(waits) - keep
                        for w in waits[:nh]:
                            ctr += 1
                            out.append({
                                "debug": ins.get("debug", 0),
                                "engine": ins["engine"],
                                "ins": [], "outs": [],
                                "name": f"lgl_wait_{ctr}",
                                "opcode": "EventSemaphore",
                                "sync_info": {"on_update": [], "on_wait": [w]},
                            })
                        si["on_wait"] = waits[nh:]
                    out.append(ins)
                blk["instructions"] = out
        return json.dumps(d).encode()

    nc.to_json_bytes = lambda: legalize(_mb.module_to_json_bytes(nc.m))
    return nc


def _build():
    nc = bass.Bass(trn_type="TRN2", target_bir_lowering=False, debug=False,
                   num_devices=NCORES)

    d_xh = nc.dram_tensor("xh", [I, RL, B], F16, kind="ExternalInput").ap()
    d_xl = nc.dram_tensor("xl", [I, RL, B], F16, kind="ExternalInput").ap()
    # W tiles: [P2][I][2 routes][CO]; each [I, 2, CO] slice is linear
    d_Wh = nc.dram_tensor("Wh", [P2, I, 2, CO], F16, kind="ExternalInput").ap()
    d_Wl = nc.dram_tensor("Wl", [P2, I, 2, CO], F16, kind="ExternalInput").ap()
    d_d0 = nc.dram_tensor("delta_s0", [128, B], F32, kind="ExternalInput").ap()
    d_d1 = nc.dram_tensor("delta_1", [128, B], F32, kind="ExternalInput").ap()
    d_vout = nc.dram_tensor("v_out", [B, CO], F32, kind="ExternalOutput").ap()

    # bounce + shared AR buffers: iter 0 has 2 halves x 4 octants; iters
    # 1,2 have 4 octants each
    d_sb0 = [[nc.dram_tensor(f"s_b0_{h}_{q}", [B, Q], F32).ap()
              for q in range(NQ)] for h in range(2)]
    d_sr0 = [[nc.dram_tensor(f"s_r0_{h}_{q}", [B, Q], F32,
                             addr_space="Shared").ap()
              for q in range(NQ)] for h in range(2)]
    d_sb = [[nc.dram_tensor(f"s_b{t}_{q}", [B, Q], F32).ap()
             for q in range(NQ)] for t in (1, 2)]
    d_sr = [[nc.dram_tensor(f"s_r{t}_{q}", [B, Q], F32,
                            addr_space="Shared").ap()
             for q in range(NQ)] for t in (1, 2)]
    d_v = [[nc.dram_tensor(f"v_{t}_{q}", [B, Q], F32).ap()
            for q in range(NQ)] for t in (0, 1)]

    groups = [list(range(NCORES))]

    with tile.TileContext(nc) as tc:
        with tc.tile_pool(name="const", bufs=1) as cpool, \
             tc.tile_pool(name="upool", bufs=1) as upool:

            t_xh = cpool.tile([I, RL * B], F16, tag="xh")
            t_xl = cpool.tile([I, RL * B], F16, tag="xl")
            nc.sync.dma_start(t_xh[:].rearrange("i (r b) -> i r b", r=RL), d_xh)
            nc.sync.dma_start(t_xl[:].rearrange("i (r b) -> i r b", r=RL), d_xl)
            t_d0 = cpool.tile([128, B], F32, tag="d0")
            t_d1 = cpool.tile([128, B], F32, tag="d1")
            nc.sync.dma_start(t_d0[:], d_d0)
            nc.sync.dma_start(t_d1[:], d_d1)
            t_eps = cpool.tile([128, 1], F32, tag="eps")
            nc.gpsimd.memset(t_eps[:], EPS)

            t_u = upool.tile([128, G, CO], F32, tag="u")

            # ---- Phase A: u_hat production + iter-0 s accumulation ----
            with tc.tile_pool(name="whp", bufs=4) as whp, \
                 tc.tile_pool(name="wlp", bufs=4) as wlp, \
                 tc.tile_pool(name="accp", bufs=1) as accp, \
                 tc.tile_pool(name="sbhp", bufs=2) as sbhp, \
                 tc.tile_pool(name="s0ps", bufs=1, space="PSUM") as s0ps, \
                 tc.tile_pool(name="prodps", bufs=3, space="PSUM") as prodps:

                s0q = [s0ps.tile([B, Q], F32, tag=f"s0q{q}", name=f"s0q{q}")
                       for q in range(NQ)]
                hi = {}
                lo = {}

                def load_hi(p):
                    if p >= P2:
                        return
                    wh = whp.tile([I, 2, CO], F16, tag="wh")
                    nc.sync.dma_start(wh[:], d_Wh[p])
                    hi[p] = wh

                def load_lo(p, h):
                    if p >= P2:
                        return
                    wl = wlp.tile([I, 2, CO // 2], F16, tag="wl")
                    nc.scalar.dma_start(
                        wl[:], d_Wl[p][:, :, (CO // 2) * h:(CO // 2) * (h + 1)])
                    lo[(p, h)] = wl

                def s0_flush_q(h, t_acc, q):
                    """rj-fold (1/C baked into d0) of one route-half's s0
                    accumulator chunk + bounce + AR."""
                    nc.tensor.matmul(s0q[q][:], t_d0[:], t_acc[:, q, :],
                                     start=True, stop=True)
                    stg = sbhp.tile([B, Q], F32, tag="sbh")
                    nc.scalar.copy(stg[:], s0q[q][:])
                    nc.sync.dma_start(d_sb0[h][q], stg[:])
                    nc.gpsimd.collective_compute(
                        "AllReduce", mybir.AluOpType.add,
                        replica_groups=groups,
                        ins=[d_sb0[h][q].opt()],
                        outs=[d_sr0[h][q].opt()])

                load_hi(0)
                load_hi(1)
                load_lo(0, 0)
                load_lo(1, 0)
                t_acc = None
                for g in range(G):
                    if g in (0, G // 2):
                        t_acc = accp.tile([128, NQ, Q], F32, tag="s0acc")
                    load_hi(2 * g + 2)
                    load_hi(2 * g + 3)
                    wA, wB = hi[2 * g], hi[2 * g + 1]
                    for q in range(NQ):
                        if q == 0:
                            load_lo(2 * g, 1)
                            load_lo(2 * g + 1, 1)
                        if q == 2:
                            load_lo(2 * g + 2, 0)
                            load_lo(2 * g + 3, 0)
                        h = q // 2
                        lA, lB = lo[(2 * g, h)], lo[(2 * g + 1, h)]
                        qq = (q % 2) * Q
                        pp = prodps.tile([128, Q], F32, tag="prod")
                        for j in range(J):
                            r = J * g + j
                            wh = wA if j < 2 else wB
                            wl = lA if j < 2 else lB
                            lane = j % 2
                            sxh = t_xh[:, r * B:(r + 1) * B]
                            sxl = t_xl[:, r * B:(r + 1) * B]
                            mvh = wh[:, lane, Q * q:Q * q + Q]
                            mvl = wl[:, lane, qq:qq + Q]
                            tp = (0, 32 * j)
                            ppj = pp[32 * j:32 * (j + 1), :]
                            nc.tensor.matmul(ppj, sxh, mvh,
                                             start=True, stop=False,
                                             tile_position=tp)
                            nc.tensor.matmul(ppj, sxl, mvh,
                                             start=False, stop=False,
                                             tile_position=tp)
                            nc.tensor.matmul(ppj, sxh, mvl,
                                             start=False, stop=True,
                                             tile_position=tp)
                        useg = t_u[:, g, Q * q:Q * q + Q]
                        accq = t_acc[:, q, :]
                        if (g + q) % 2 == 0:
                            nc.vector.tensor_copy(useg, pp[:])
                        else:
                            nc.scalar.copy(useg, pp[:])
                        if g in (0, G // 2):
                            nc.scalar.copy(accq, pp[:])
                        else:
                            nc.vector.tensor_add(accq, accq, pp[:])
                        if g == G - 1:
                            # flush each half-1 s0 chunk the moment its
                            # accumulator takes the final add
                            s0_flush_q(1, t_acc, q)
                    if g == G // 2 - 1:
                        for q in range(NQ):
                            s0_flush_q(0, t_acc, q)

            # ---- Phase B: routing iterations ----
            with tc.tile_pool(name="iter", bufs=1) as ip, \
                 tc.tile_pool(name="tmp", bufs=2) as tp_pool, \
                 tc.tile_pool(name="sps", bufs=1, space="PSUM") as sps:

                t_vrep = ip.tile([128, CO], F32, tag="vrep")
                t_b = ip.tile([128, G, C], F32, tag="bij")
                t_a = ip.tile([128, G, C], F32, tag="aij")
                t_cij = ip.tile([128, G, C], F32, tag="cij")
                t_mx = ip.tile([128, G], F32, tag="mx")
                t_rs = ip.tile([128, G], F32, tag="rs")
                t_sbounce = ip.tile([B, CO], F32, tag="sbounce")
                t_spk = ip.tile([128, NQ, 2 * O], F32, tag="spk")
                t_sq = ip.tile([128, 2 * O], F32, tag="sqt")
                t_rt = ip.tile([128, 2 * O], F32, tag="rt")
                t_vpk = ip.tile([128, NQ, 2 * O], F32, tag="vpk")

                u4 = t_u[:].rearrange("p g (c o) -> p g c o", c=C)
                vr4 = t_vrep[:].rearrange("p (c o) -> p c o", c=C)

                def spk_load(src, q):
                    """Pack one AR output octant into t_spk[:, q, :]."""
                    spk = t_spk[:, q, :]
                    for c4 in range(4):
                        dst = spk.rearrange("p (c2 o) -> p c2 o", c2=2) \
                            [32 * c4:32 * (c4 + 1)]
                        srcv = src.rearrange("b (c8 o) -> b c8 o", o=O)
                        nc.sync.dma_start(dst, srcv[:, 2 * c4:2 * c4 + 2, :])

                def squash_chunk(srcs, q, vdst, preloaded=False):
                    """AR outputs (1-2 DRAM [B, Q] bufs to sum) -> packed
                    squash -> v octant to DRAM vdst."""
                    spk = t_spk[:, q, :]
                    vpk = t_vpk[:, q, :]
                    if not preloaded:
                        spk_load(srcs[0], q)
                    if len(srcs) > 1:
                        for c4 in range(4):
                            dst = t_sq[:].rearrange("p (c2 o) -> p c2 o", c2=2) \
                                [32 * c4:32 * (c4 + 1)]
                            src1 = srcs[1].rearrange("b (c8 o) -> b c8 o", o=O)
                            nc.sync.dma_start(
                                dst, src1[:, 2 * c4:2 * c4 + 2, :])
                        nc.vector.tensor_add(spk, spk, t_sq[:])
                    nc.scalar.square(t_sq[:], spk)
                    nc.scalar.activation(t_rt[:], t_sq[:],
                                         mybir.ActivationFunctionType.Sqrt,
                                         bias=t_eps[:])
                    nc.vector.tensor_scalar_add(vpk, t_sq[:], 1.0)
                    nc.vector.tensor_mul(t_rt[:], t_rt[:], vpk)
                    nc.vector.reciprocal(t_rt[:], t_rt[:])
                    nc.vector.tensor_mul(t_sq[:], t_sq[:], spk)
                    nc.vector.tensor_mul(vpk, t_sq[:], t_rt[:])
                    for c4 in range(4):
                        src = vpk.rearrange("p (c2 o) -> p c2 o", c2=2) \
                            [32 * c4:32 * (c4 + 1)]
                        dst = vdst.rearrange("b (c8 o) -> b c8 o", o=O)
                        nc.sync.dma_start(dst[:, 2 * c4:2 * c4 + 2, :], src)

                def vrep_chunk(vsrc, q):
                    for j in range(J):
                        nc.sync.dma_start(
                            t_vrep[32 * j:32 * (j + 1), Q * q:Q * q + Q],
                            vsrc)

                def a_chunk(cc, first):
                    cs = slice(4 * cc, 4 * cc + 4)
                    eng = nc.gpsimd if cc >= 6 else nc.vector
                    dst = t_b if first else t_a
                    tmp = tp_pool.tile([128, G, 4, O], F32, tag="tmp")
                    in1 = vr4[:, cs, :].unsqueeze(1) \
                        .broadcast_to([128, G, 4, O])
                    eng.tensor_tensor(tmp[:], u4[:, :, cs, :], in1,
                                      mybir.AluOpType.mult)
                    nc.vector.tensor_reduce(dst[:, :, cs], tmp[:],
                                            axis=mybir.AxisListType.X,
                                            op=mybir.AluOpType.add)

                def softmax(first):
                    if not first:
                        nc.vector.tensor_add(t_b[:], t_b[:], t_a[:])
                    nc.vector.tensor_reduce(t_mx[:], t_b[:],
                                            axis=mybir.AxisListType.X,
                                            op=mybir.AluOpType.max)
                    mxb = t_mx[:].unsqueeze(2).broadcast_to([128, G, C])
                    nc.vector.tensor_sub(t_cij[:], t_b[:], mxb)
                    nc.scalar.activation(t_cij[:], t_cij[:],
                                         mybir.ActivationFunctionType.Exp)
                    nc.vector.tensor_reduce(t_rs[:], t_cij[:],
                                            axis=mybir.AxisListType.X,
                                            op=mybir.AluOpType.add)
                    nc.vector.reciprocal(t_rs[:], t_rs[:])
                    rsb = t_rs[:].unsqueeze(2).broadcast_to([128, G, C])
                    nc.vector.tensor_mul(t_cij[:], t_cij[:], rsb)

                def s_octant(q, t):
                    """c*u mult (DVE, 2 half-octants) + route-fold on
                    TensorE into one PSUM bank + AR launch."""
                    pq = sps.tile([B, Q], F32, tag=f"sq{q}",
                                  name=f"sq{q}_{t}")
                    for hh in range(2):
                        cs = slice(8 * q + 4 * hh, 8 * q + 4 * hh + 4)
                        tmp = tp_pool.tile([128, G, 4, O], F32, tag="tmp")
                        in1 = t_cij[:, :, cs].unsqueeze(3) \
                            .broadcast_to([128, G, 4, O])
                        nc.vector.tensor_tensor(tmp[:], u4[:, :, cs, :], in1,
                                                mybir.AluOpType.mult)
                        tmpf = tmp[:].rearrange("p g c o -> p g (c o)")
                        out = pq[:, HQ * hh:HQ * hh + HQ]
                        for g in range(G):
                            nc.tensor.matmul(out, t_d1[:], tmpf[:, g, :],
                                             start=(g == 0),
                                             stop=(g == G - 1))
                    nc.scalar.copy(t_sbounce[:, Q * q:Q * q + Q], pq[:])
                    nc.sync.dma_start(d_sb[t - 1][q],
                                      t_sbounce[:, Q * q:Q * q + Q])
                    nc.gpsimd.collective_compute(
                        "AllReduce", mybir.AluOpType.add,
                        replica_groups=groups,
                        ins=[d_sb[t - 1][q].opt()],
                        outs=[d_sr[t - 1][q].opt()])

                # --- iteration 0 tail: half-0 AR results are long done —
                # preload them all, then per-octant: add half-1 + squash +
                # broadcast + first a-pass ---
                for q in range(NQ):
                    spk_load(d_sr0[0][q], q)
                for q in range(NQ):
                    squash_chunk([d_sr0[0][q], d_sr0[1][q]], q, d_v[0][q],
                                 preloaded=True)
                    vrep_chunk(d_v[0][q], q)
                    a_chunk(2 * q, first=True)
                    a_chunk(2 * q + 1, first=True)

                # --- iteration 1 ---
                softmax(first=True)
                for q in range(NQ):
                    s_octant(q, t=1)
                for q in range(NQ):
                    squash_chunk([d_sr[0][q]], q, d_v[1][q])
                    vrep_chunk(d_v[1][q], q)
                    a_chunk(2 * q, first=False)
                    a_chunk(2 * q + 1, first=False)

                # --- iteration 2 ---
                softmax(first=False)
                for q in range(NQ):
                    s_octant(q, t=2)
                for q in range(NQ):
                    squash_chunk([d_sr[1][q]], q,
                                 d_vout[:, Q * q:Q * q + Q])

    _legalize_install(nc)
    return nc


def _prep_inputs(x, W):
    x_t = np.ascontiguousarray(x.transpose(2, 1, 0))          # [I, R, B]
    xh = x_t.astype(np.float16)
    xl = (x_t - xh.astype(np.float32)).astype(np.float16)
    # W [R, C, O, I] -> per-core [P2, I, 2, CO] (2 routes per tile)
    W_t = np.ascontiguousarray(W.transpose(0, 3, 1, 2)).reshape(R, I, CO)
    Wh = W_t.astype(np.float16)
    Wl = (W_t - Wh.astype(np.float32)).astype(np.float16)
    d0 = np.tile(np.eye(B, dtype=np.float32) / C, (J, 1))
    d1 = np.tile(np.eye(B, dtype=np.float32), (J, 1))
    in_maps = []
    for k in range(NCORES):
        rk = slice(RL * k, RL * (k + 1))
        wh_k = np.ascontiguousarray(
            Wh[rk].reshape(P2, 2, I, CO).transpose(0, 2, 1, 3))
        wl_k = np.ascontiguousarray(
            Wl[rk].reshape(P2, 2, I, CO).transpose(0, 2, 1, 3))
        in_maps.append({
            "xh": np.ascontiguousarray(xh[:, rk, :]),
            "xl": np.ascontiguousarray(xl[:, rk, :]),
            "Wh": wh_k, "Wl": wl_k,
            "delta_s0": d0, "delta_1": d1,
        })
    return in_maps


def kernel(x: np.ndarray, W: np.ndarray, **run_kwargs) -> np.ndarray:
    if "nc" not in _cache:
        _cache["nc"] = _build()
    nc = _cache["nc"]
    in_maps = _prep_inputs(np.asarray(x), np.asarray(W))
    res = run_bass_kernel_spmd(nc, in_maps, core_ids=list(range(NCORES)),
                               **run_kwargs)
    v = res.results[0]["v_out"].reshape(B, C, O, 1).astype(np.float32)
    if run_kwargs:
        _cache["last_results"] = res
    return v
